# revision 6
# baseline (speedup 1.0000x reference)
"""MoE gate (group-limited greedy routing) on 8 Trainium2 NeuronCores.

Math (per token t):
    logits = x[t, 1:] @ weight.T                    (64 experts)
    scores = sigmoid(logits)
    sb     = scores + bias
    group_scores[g] = sum(top2(sb[g*8:(g+1)*8]))    (8 groups)
    keep top-4 groups; mask the rest to -inf
    top-8 experts of masked sb -> indices
    weights = 2.5 * normalize(scores[indices])

Device strategy per core (4096 tokens):
  - host passes x[:, 1:].T  (feature-major, zero-padded to 2048 rows) so the
    contraction dim lands on partitions with contiguous DMA runs.
  - weight-stationary fp32 matmul: lhsT = wT k-tile [128, 64],
    rhs = xT k-tile [128, 512] -> psum [64 experts, 512 tokens], 16 k-tiles.
  - PE transpose (identity matmul) back to [128 tokens, 64 experts].
  - sigmoid on ACT; group top-2 / top-4 / top-8 via the DVE max8 unit;
    ordered score gather via (masked == top8_value) * scores with fused
    per-partition accumulate (scalar_tensor_tensor accum_out).
"""

import sys

sys.path.insert(0, "/opt/trn_rl_repo")

import numpy as np
import concourse.bacc as bacc
import concourse.mybir as mybir
from concourse.tile import TileContext
from concourse.bass_utils import run_bass_kernel_spmd

F32 = mybir.dt.float32
U32 = mybir.dt.uint32
I32 = mybir.dt.int32
Alu = mybir.AluOpType
Act = mybir.ActivationFunctionType

T = 32768
DIM = 2048
E = 64
G = 8
GS = E // G          # 8 experts per group
TOPK = 8
ROUTE_SCALE = 2.5

NCORES = 8
TPC = T // NCORES    # 4096 tokens per core
CHUNK = 512          # tokens per matmul chunk
NCHUNK = TPC // CHUNK
KP = 128             # contraction tile
KT = DIM // KP       # 16 k-tiles (feature dim padded 2047 -> 2048)

NEG = -1.0e9

_CACHE = {}


def _topk_tile(nc, pool, sc, br_sb, negc, w_out, i_out, row0):
    """Group-limited top-8 for one [128 tokens, 64 experts] score tile."""
    P = 128
    sb = pool.tile([P, E], F32, tag="sb")
    nc.vector.tensor_add(sb[:], sc[:], br_sb[:])

    # top-8 (sorted) of each 8-expert group -> top-2 sum per group
    gtop = pool.tile([P, E], F32, tag="gtop")
    for g in range(G):
        nc.vector.max(out=gtop[:, g * GS:(g + 1) * GS],
                      in_=sb[:, g * GS:(g + 1) * GS])
    gs = pool.tile([P, G], F32, tag="gs")
    gv = gtop[:].rearrange("p (g s) -> p g s", s=GS)
    nc.vector.tensor_add(gs[:], gv[:, :, 0], gv[:, :, 1])

    # threshold = 4th largest group score; penalty -1e9 for dropped groups
    g8 = pool.tile([P, 8], F32, tag="g8")
    nc.vector.max(out=g8[:], in_=gs[:])
    pen = pool.tile([P, G], F32, tag="pen")
    nc.vector.scalar_tensor_tensor(
        out=pen[:], in0=gs[:], scalar=g8[:, 3:4], in1=negc[:],
        op0=Alu.is_lt, op1=Alu.mult)

    mk = pool.tile([P, E], F32, tag="mk")
    nc.vector.tensor_add(
        mk[:].rearrange("p (g s) -> p g s", s=GS),
        sb[:].rearrange("p (g s) -> p g s", s=GS),
        pen[:].unsqueeze(2).to_broadcast([P, G, GS]))

    # top-8 experts of masked sb (values sorted desc + their indices)
    v8 = pool.tile([P, 8], F32, tag="v8")
    nc.vector.max(out=v8[:], in_=mk[:])
    ix = pool.tile([P, 8], U32, tag="ix")
    nc.vector.max_index(out=ix[:], in_max=v8[:], in_values=mk[:])

    # ordered gather of original scores: (mk == v8[j]) * scores, summed
    gat = pool.tile([P, 8], F32, tag="gat")
    junk = pool.tile([P, E], F32, tag="junk")
    for j in range(TOPK):
        nc.vector.scalar_tensor_tensor(
            out=junk[:], in0=mk[:], scalar=v8[:, j:j + 1], in1=sc[:],
            op0=Alu.is_equal, op1=Alu.mult, accum_out=gat[:, j:j + 1])

    # normalize * 2.5
    s1 = pool.tile([P, 1], F32, tag="s1")
    nc.vector.tensor_reduce(s1[:], gat[:], axis=mybir.AxisListType.X, op=Alu.add)
    r1 = pool.tile([P, 1], F32, tag="r1")
    nc.vector.reciprocal(r1[:], s1[:])
    wo = pool.tile([P, 8], F32, tag="wo")
    nc.vector.tensor_scalar(
        out=wo[:], in0=gat[:], scalar1=r1[:, 0:1], scalar2=float(ROUTE_SCALE),
        op0=Alu.mult, op1=Alu.mult)

    nc.sync.dma_start(w_out[row0:row0 + P, :], wo[:])
    nc.sync.dma_start(i_out[row0:row0 + P, :], ix[:].bitcast(I32))


def _build_nc(n_repeat=1):
    nc = bacc.Bacc(None, target_bir_lowering=False, debug=False)

    xt = nc.declare_dram_parameter("xt", [KT * KP, TPC], F32, isOutput=False)
    wt = nc.declare_dram_parameter("wt", [KT * KP, E], F32, isOutput=False)
    br = nc.declare_dram_parameter("br", [128, E], F32, isOutput=False)
    idn = nc.declare_dram_parameter("idn", [E, E], F32, isOutput=False)
    w_out = nc.declare_dram_parameter("w_out", [TPC, TOPK], F32, isOutput=True)
    i_out = nc.declare_dram_parameter("i_out", [TPC, TOPK], I32, isOutput=True)

    with TileContext(nc) as tc:
        with (
            tc.tile_pool(name="const", bufs=1) as cpool,
            tc.tile_pool(name="xts", bufs=36) as xpool,
            tc.tile_pool(name="work", bufs=4) as wpool,
            tc.tile_pool(name="psmm", bufs=2, space="PSUM") as psA,
            tc.tile_pool(name="pstr", bufs=4, space="PSUM") as psB,
        ):
            # constants: 16 wT k-tiles side by side, bias row-replicated,
            # 64x64 identity, -1e9
            wt_sb = cpool.tile([KP, KT * E], F32)
            nc.sync.dma_start(
                wt_sb[:].rearrange("p (k e) -> p k e", k=KT),
                wt[:, :].rearrange("(k p) e -> p k e", p=KP))
            br_sb = cpool.tile([128, E], F32)
            nc.sync.dma_start(br_sb[:], br[:, :])
            id_sb = cpool.tile([E, E], F32)
            nc.sync.dma_start(id_sb[:], idn[:, :])
            negc = cpool.tile([128, G], F32)
            nc.vector.memset(negc[:], NEG)

            if n_repeat > 1:
                import contextlib
                rep_ctx = tc.For_i(0, n_repeat, 1)
            else:
                import contextlib
                rep_ctx = contextlib.nullcontext()
            with rep_ctx:
                _body(nc, tc, cpool, xpool, wpool, psA, psB,
                      xt, w_out, i_out, wt_sb, br_sb, id_sb, negc)

    nc.compile()
    return nc


def _body(nc, tc, cpool, xpool, wpool, psA, psB,
          xt, w_out, i_out, wt_sb, br_sb, id_sb, negc):
    if True:
        if True:
            for c in range(NCHUNK):
                t0 = c * CHUNK
                xk = []
                for k in range(KT):
                    tl = xpool.tile([KP, CHUNK], F32, tag="xt")
                    nc.sync.dma_start(
                        tl[:], xt[k * KP:(k + 1) * KP, t0:t0 + CHUNK])
                    xk.append(tl)

                ps = psA.tile([E, CHUNK], F32, tag="mm")
                for k in range(KT):
                    nc.tensor.matmul(
                        ps[:], wt_sb[:, k * E:(k + 1) * E], xk[k][:],
                        start=(k == 0), stop=(k == KT - 1))

                lg = wpool.tile([E, CHUNK], F32, tag="lg")
                nc.vector.tensor_copy(lg[:], ps[:])

                for j in range(CHUNK // 128):
                    pt = psB.tile([128, E], F32, tag="pt")
                    nc.tensor.transpose(
                        pt[:], lg[:, j * 128:(j + 1) * 128], id_sb[:])
                    sc = wpool.tile([128, E], F32, tag="sc")
                    nc.scalar.activation(sc[:], pt[:], Act.Sigmoid)
                    _topk_tile(nc, wpool, sc, br_sb, negc,
                               w_out, i_out, t0 + j * 128)


def _get_nc():
    if "nc" not in _CACHE:
        _CACHE["nc"] = _build_nc()
    return _CACHE["nc"]


def _prep_inputs(x, weight, bias):
    x = np.asarray(x, dtype=np.float32)
    weight = np.asarray(weight, dtype=np.float32)
    bias = np.asarray(bias, dtype=np.float32)
    assert x.shape == (T, DIM) and weight.shape == (E, DIM - 1)

    wt = np.zeros((KT * KP, E), dtype=np.float32)
    wt[:DIM - 1] = weight.T
    br = np.tile(bias[None, :], (128, 1)).astype(np.float32)
    idn = np.eye(E, dtype=np.float32)

    in_maps = []
    for c in range(NCORES):
        xt = np.zeros((KT * KP, TPC), dtype=np.float32)
        xt[:DIM - 1] = x[c * TPC:(c + 1) * TPC, 1:].T
        in_maps.append({"xt": xt, "wt": wt, "br": br, "idn": idn})
    return in_maps


def kernel(x, weight, bias):
    nc = _get_nc()
    in_maps = _prep_inputs(x, weight, bias)
    out = run_bass_kernel_spmd(nc, in_maps, list(range(NCORES)))
    _CACHE["last_result"] = out
    res = out.results
    weights = np.concatenate([res[c]["w_out"] for c in range(NCORES)], axis=0)
    indices = np.concatenate([res[c]["i_out"] for c in range(NCORES)], axis=0)
    return weights, indices


# ---------------------------------------------------------------------------
# benchmarking helpers (not used by the grader; kernel() above is the entry)
# ---------------------------------------------------------------------------

def _timed_runner(nc, in_maps):
    """Mirror bass2jax.run_bass_via_pjrt's multi-core path, but keep inputs
    resident on device and return a closure that runs + blocks."""
    import jax
    from jax.sharding import Mesh, PartitionSpec, NamedSharding
    from jax.experimental.shard_map import shard_map
    from concourse import bass2jax

    bass2jax.install_neuronx_cc_hook()
    if nc.dbg_addr is not None:
        in_maps = [
            {**m, nc.dbg_addr.name: np.zeros((1, 2), np.uint32)} for m in in_maps
        ]
    partition_name = nc.partition_id_tensor.name if nc.partition_id_tensor else None
    in_names, out_names, out_avals, zero_outs = [], [], [], []
    for alloc in nc.m.functions[0].allocations:
        if not isinstance(alloc, mybir.MemoryLocationSet):
            continue
        name = alloc.memorylocations[0].name
        if alloc.kind == "ExternalInput":
            if name != partition_name:
                in_names.append(name)
        elif alloc.kind == "ExternalOutput":
            shape = tuple(alloc.tensor_shape)
            dtype = mybir.dt.np(alloc.dtype)
            out_names.append(name)
            out_avals.append(jax.core.ShapedArray(shape, dtype))
            zero_outs.append(np.zeros(shape, dtype))
    n_params = len(in_names)
    n_cores = len(in_maps)
    all_in_names = list(in_names) + list(out_names)
    if partition_name is not None:
        all_in_names.append(partition_name)

    def _b(*args):
        operands = list(args)
        if partition_name is not None:
            operands.append(bass2jax.partition_id_tensor())
        outs = bass2jax._bass_exec_p.bind(
            *operands,
            out_avals=tuple(out_avals),
            in_names=tuple(all_in_names),
            out_names=tuple(out_names),
            lowering_input_output_aliases=(),
            sim_require_finite=True,
            sim_require_nnan=True,
            nc=nc,
        )
        return tuple(outs)

    devices = jax.devices()[:n_cores]
    mesh = Mesh(np.asarray(devices), ("core",))
    in_specs = (PartitionSpec("core"),) * (n_params + len(out_names))
    out_specs = (PartitionSpec("core"),) * len(out_names)
    fn = jax.jit(shard_map(_b, mesh=mesh, in_specs=in_specs,
                           out_specs=out_specs, check_rep=False))
    sh = NamedSharding(mesh, PartitionSpec("core"))
    concat_in = [
        jax.device_put(
            np.concatenate([np.asarray(in_maps[c][nm]) for c in range(n_cores)], 0),
            sh)
        for nm in in_names
    ]
    concat_zeros = [
        jax.device_put(np.zeros((n_cores * z.shape[0], *z.shape[1:]), z.dtype), sh)
        for z in zero_outs
    ]

    def run():
        outs = fn(*concat_in, *concat_zeros)
        jax.block_until_ready(outs)
        return outs

    return run


def bench(x, weight, bias, n_repeat=32, trials=12):
    """Returns per-invocation HW-ish ns: (t[rep=R] - t[rep=1]) / (R - 1)."""
    import time
    in_maps = _prep_inputs(x, weight, bias)
    results = {}
    for rep in (1, n_repeat):
        key = f"nc_rep{rep}"
        if key not in _CACHE:
            _CACHE[key] = _build_nc(rep)
        run = _timed_runner(_CACHE[key], in_maps)
        run()  # warmup / compile
        ts = []
        for _ in range(trials):
            t0 = time.perf_counter()
            run()
            ts.append(time.perf_counter() - t0)
        results[rep] = min(ts)
        print(f"  repeat={rep}: min exec {min(ts)*1e3:.3f} ms "
              f"(median {sorted(ts)[len(ts)//2]*1e3:.3f} ms)")
    ns = (results[n_repeat] - results[1]) / (n_repeat - 1) * 1e9
    return ns
